# revision 2
# baseline (speedup 1.0000x reference)
"""Block-sparse linear y = x @ W^T + b on 8 Trainium2 NeuronCores.

x: [8192, 4096] f32, W: [4096, 4096] f32 (block-sparse mask already applied),
b: [4096] f32 -> y: [8192, 4096] f32.

Sharding: 2 row-halves of x  x  4 out-feature quarters of W (8 cores).
Each core computes y_shard[4096, 1024] = x_half @ W_quarter^T + b_quarter
with fp32r matmuls (full-rate fp32 on the PE array, ~1.5e-4 rel err),
W-shard resident in SBUF, accumulating over 32 K-tiles in PSUM.
"""

import numpy as np
import jax
from jax.sharding import Mesh, PartitionSpec
from jax.experimental.shard_map import shard_map

import concourse.bass as bass
import concourse.tile as tile
from concourse import bacc, mybir
from concourse.bass2jax import (
    install_neuronx_cc_hook,
    partition_id_tensor,
    _bass_exec_p,
)

P = 128
K = 4096          # contraction (in_features)
N_ROWS = 8192     # x rows
D_OUT = 4096      # out_features
R_SHARDS = 2      # row shards
C_SHARDS = 4      # out-feature shards
MC = N_ROWS // R_SHARDS    # 4096 rows per core
NC_ = D_OUT // C_SHARDS    # 1024 out features per core
KT = K // P                # 32 k-tiles
MT = MC // P               # 32 row-tiles
NT = NC_ // 512            # 2 psum-width tiles

F32 = mybir.dt.float32
F32R = mybir.dt.float32r

_CACHE = {}


def _build_nc():
    nc = bacc.Bacc("TRN2", target_bir_lowering=False)
    xt_d = nc.declare_dram_parameter("xt", [K, MC], F32, isOutput=False).ap()
    wt_d = nc.declare_dram_parameter("wt", [K, NC_], F32, isOutput=False).ap()
    b_d = nc.declare_dram_parameter("bias", [P, NC_], F32, isOutput=False).ap()
    y_d = nc.declare_dram_parameter("y", [MC, NC_], F32, isOutput=True).ap()

    with tile.TileContext(nc) as tc:
        with (
            tc.tile_pool(name="wpool", bufs=1) as wpool,
            tc.tile_pool(name="bpool", bufs=1) as bpool,
            tc.tile_pool(name="xpool", bufs=3) as xpool,
            tc.tile_pool(name="opool", bufs=2) as opool,
            tc.tile_pool(name="psum", bufs=4, space="PSUM") as psum,
        ):
            # resident weights [p, kt, NC_]; split the load per k-tile so
            # early matmuls can start before the whole shard arrives
            wt_sb = wpool.tile([P, KT, NC_], F32R)
            wt_src = wt_d.rearrange("(j p) n -> p j n", p=P).bitcast(F32R)
            for j in range(KT):
                nc.sync.dma_start(out=wt_sb[:, j, :], in_=wt_src[:, j, :])
            b_sb = bpool.tile([P, NC_], F32)
            nc.sync.dma_start(out=b_sb[:], in_=b_d[:])

            for m in range(MT):
                xt_sb = xpool.tile([P, KT, P], F32R)
                nc.sync.dma_start(
                    out=xt_sb[:],
                    in_=xt_d[:, m * P : (m + 1) * P]
                    .rearrange("(j p) f -> p j f", p=P)
                    .bitcast(F32R),
                )
                o_sb = opool.tile([P, NC_], F32)
                for n in range(NT):
                    ps = psum.tile([P, 512], F32)
                    for j in range(KT):
                        nc.tensor.matmul(
                            ps[:],
                            lhsT=xt_sb[:, j, :],
                            rhs=wt_sb[:, j, n * 512 : (n + 1) * 512],
                            start=(j == 0),
                            stop=(j == KT - 1),
                        )
                    nc.vector.tensor_add(
                        out=o_sb[:, n * 512 : (n + 1) * 512],
                        in0=ps[:],
                        in1=b_sb[:, n * 512 : (n + 1) * 512],
                    )
                nc.sync.dma_start(out=y_d[m * P : (m + 1) * P, :], in_=o_sb[:])
    nc.compile()
    return nc


def _get_runner():
    """Build (once) a jitted 8-core executable: concat inputs -> concat outputs."""
    if "runner" in _CACHE:
        return _CACHE["runner"]

    install_neuronx_cc_hook()
    nc = _build_nc()

    partition_name = (
        nc.partition_id_tensor.name if nc.partition_id_tensor else None
    )
    in_names = []
    out_names = []
    out_avals = []
    out_shapes = []
    for alloc in nc.m.functions[0].allocations:
        if not isinstance(alloc, mybir.MemoryLocationSet):
            continue
        name = alloc.memorylocations[0].name
        if alloc.kind == "ExternalInput":
            if name != partition_name:
                in_names.append(name)
        elif alloc.kind == "ExternalOutput":
            shape = tuple(alloc.tensor_shape)
            out_names.append(name)
            out_shapes.append(shape)
            out_avals.append(
                jax.core.ShapedArray(shape, mybir.dt.np(alloc.dtype))
            )
    n_params = len(in_names)
    # outputs are passed as (non-donated) zero operands after the inputs
    all_names = in_names + out_names
    if partition_name is not None:
        all_names = all_names + [partition_name]

    def _body(*args):
        operands = list(args)
        if partition_name is not None:
            operands.append(partition_id_tensor())
        outs = _bass_exec_p.bind(
            *operands,
            out_avals=tuple(out_avals),
            in_names=tuple(all_names),
            out_names=tuple(out_names),
            lowering_input_output_aliases=(),
            sim_require_finite=True,
            sim_require_nnan=True,
            nc=nc,
        )
        return tuple(outs)

    devices = jax.devices()[:8]
    mesh = Mesh(np.asarray(devices), ("core",))
    n_outs = len(out_names)
    sharded = jax.jit(
        shard_map(
            _body,
            mesh=mesh,
            in_specs=(PartitionSpec("core"),) * (n_params + n_outs),
            out_specs=(PartitionSpec("core"),) * n_outs,
            check_rep=False,
        ),
        keep_unused=True,
    )
    runner = {
        "fn": sharded,
        "in_names": in_names,
        "out_names": out_names,
        "out_shapes": out_shapes,
    }
    _CACHE["runner"] = runner
    return runner


def _run_cores(in_maps):
    """in_maps: list of 8 dicts name->np.ndarray. Returns list of 8 output dicts."""
    r = _get_runner()
    concat_in = [
        np.concatenate([np.asarray(m[name]) for m in in_maps], axis=0)
        for name in r["in_names"]
    ]
    concat_zeros = [
        np.zeros((8 * s[0], *s[1:]), np.float32) for s in r["out_shapes"]
    ]
    out_arrs = r["fn"](*concat_in, *concat_zeros)
    outs = []
    for c in range(8):
        outs.append(
            {
                name: np.asarray(out_arrs[i]).reshape(8, *r["out_shapes"][i])[c]
                for i, name in enumerate(r["out_names"])
            }
        )
    return outs


def _make_in_maps(x, weight, bias):
    xt = np.ascontiguousarray(np.asarray(x, dtype=np.float32).T)      # [K, N_ROWS]
    wt = np.ascontiguousarray(np.asarray(weight, dtype=np.float32).T)  # [K, D_OUT]
    bias = np.asarray(bias, dtype=np.float32)
    in_maps = []
    for i in range(8):
        h, q = divmod(i, C_SHARDS)
        in_maps.append(
            {
                "xt": xt[:, h * MC : (h + 1) * MC],
                "wt": wt[:, q * NC_ : (q + 1) * NC_],
                "bias": np.broadcast_to(
                    bias[q * NC_ : (q + 1) * NC_], (P, NC_)
                ),
            }
        )
    return in_maps


def kernel(x, weight, bias):
    in_maps = _make_in_maps(x, weight, bias)
    outs = _run_cores(in_maps)
    y = np.empty((N_ROWS, D_OUT), dtype=np.float32)
    for i in range(8):
        h, q = divmod(i, C_SHARDS)
        y[h * MC : (h + 1) * MC, q * NC_ : (q + 1) * NC_] = outs[i]["y"]
    return y


# revision 3
# speedup vs baseline: 134.5234x; 134.5234x over previous
"""Block-sparse linear y = x @ W^T + b on 8 Trainium2 NeuronCores.

x: [8192, 4096] f32, W: [4096, 4096] f32 (block-sparse mask already applied),
b: [4096] f32 -> y: [8192, 4096] f32.

Sharding: 2 row-halves of x  x  4 out-feature quarters of W (8 cores).
Each core computes y_shard[4096, 1024] = x_half @ W_quarter^T + b_quarter
with fp32r matmuls (full-rate fp32 on the PE array, ~1.5e-4 rel err),
W-shard resident in SBUF, accumulating over 32 K-tiles in PSUM.
"""

import numpy as np
import jax
from jax.sharding import Mesh, PartitionSpec
from jax.experimental.shard_map import shard_map

import concourse.bass as bass
import concourse.tile as tile
from concourse import bacc, mybir
from concourse.bass2jax import (
    install_neuronx_cc_hook,
    partition_id_tensor,
    _bass_exec_p,
)

P = 128
K = 4096          # contraction (in_features)
N_ROWS = 8192     # x rows
D_OUT = 4096      # out_features
R_SHARDS = 2      # row shards
C_SHARDS = 4      # out-feature shards
MC = N_ROWS // R_SHARDS    # 4096 rows per core
NC_ = D_OUT // C_SHARDS    # 1024 out features per core
KT = K // P                # 32 k-tiles
MT = MC // P               # 32 row-tiles
NT = NC_ // 512            # 2 psum-width tiles

F32 = mybir.dt.float32
F32R = mybir.dt.float32r

_CACHE = {}


def _build_nc(repeats=1):
    nc = bacc.Bacc("TRN2", target_bir_lowering=False)
    xt_d = nc.declare_dram_parameter("xt", [K, MC], F32, isOutput=False).ap()
    wt_d = nc.declare_dram_parameter("wt", [K, NC_], F32, isOutput=False).ap()
    b_d = nc.declare_dram_parameter("bias", [P, NC_], F32, isOutput=False).ap()
    y_d = nc.declare_dram_parameter("y", [MC, NC_], F32, isOutput=True).ap()

    with tile.TileContext(nc) as tc:
        with (
            tc.tile_pool(name="wpool", bufs=1) as wpool,
            tc.tile_pool(name="bpool", bufs=1) as bpool,
            tc.tile_pool(name="xpool", bufs=3) as xpool,
            tc.tile_pool(name="opool", bufs=2) as opool,
            tc.tile_pool(name="psum", bufs=4, space="PSUM") as psum,
        ):
            # resident weights [p, kt, NC_]; split the load per k-tile so
            # early matmuls can start before the whole shard arrives
            wt_sb = wpool.tile([P, KT, NC_], F32R)
            wt_src = wt_d.rearrange("(j p) n -> p j n", p=P).bitcast(F32R)
            for j in range(KT):
                nc.sync.dma_start(out=wt_sb[:, j, :], in_=wt_src[:, j, :])
            b_sb = bpool.tile([P, NC_], F32)
            nc.sync.dma_start(out=b_sb[:], in_=b_d[:])

            import contextlib
            rep_ctx = (
                tc.For_i(0, repeats, 1, hint_engines=(mybir.EngineType.PE,))
                if repeats > 1
                else contextlib.nullcontext()
            )
            with rep_ctx:
                _emit_body(nc, tc, xpool, opool, psum, xt_d, y_d, wt_sb, b_sb)
    nc.compile()
    return nc


def _emit_body(nc, tc, xpool, opool, psum, xt_d, y_d, wt_sb, b_sb):
    if True:
            for m in range(MT):
                xt_sb = xpool.tile([P, KT, P], F32R)
                nc.sync.dma_start(
                    out=xt_sb[:],
                    in_=xt_d[:, m * P : (m + 1) * P]
                    .rearrange("(j p) f -> p j f", p=P)
                    .bitcast(F32R),
                )
                o_sb = opool.tile([P, NC_], F32)
                for n in range(NT):
                    ps = psum.tile([P, 512], F32)
                    for j in range(KT):
                        nc.tensor.matmul(
                            ps[:],
                            lhsT=xt_sb[:, j, :],
                            rhs=wt_sb[:, j, n * 512 : (n + 1) * 512],
                            start=(j == 0),
                            stop=(j == KT - 1),
                        )
                    nc.vector.tensor_add(
                        out=o_sb[:, n * 512 : (n + 1) * 512],
                        in0=ps[:],
                        in1=b_sb[:, n * 512 : (n + 1) * 512],
                    )
                nc.sync.dma_start(out=y_d[m * P : (m + 1) * P, :], in_=o_sb[:])


def _get_runner(repeats=1):
    """Build (once) a jitted 8-core executable: concat inputs -> concat outputs."""
    key = ("runner", repeats)
    if key in _CACHE:
        return _CACHE[key]

    install_neuronx_cc_hook()
    nc = _build_nc(repeats)

    partition_name = (
        nc.partition_id_tensor.name if nc.partition_id_tensor else None
    )
    in_names = []
    out_names = []
    out_avals = []
    out_shapes = []
    for alloc in nc.m.functions[0].allocations:
        if not isinstance(alloc, mybir.MemoryLocationSet):
            continue
        name = alloc.memorylocations[0].name
        if alloc.kind == "ExternalInput":
            if name != partition_name:
                in_names.append(name)
        elif alloc.kind == "ExternalOutput":
            shape = tuple(alloc.tensor_shape)
            out_names.append(name)
            out_shapes.append(shape)
            out_avals.append(
                jax.core.ShapedArray(shape, mybir.dt.np(alloc.dtype))
            )
    n_params = len(in_names)
    # outputs are passed as (non-donated) zero operands after the inputs
    all_names = in_names + out_names
    if partition_name is not None:
        all_names = all_names + [partition_name]

    def _body(*args):
        operands = list(args)
        if partition_name is not None:
            operands.append(partition_id_tensor())
        outs = _bass_exec_p.bind(
            *operands,
            out_avals=tuple(out_avals),
            in_names=tuple(all_names),
            out_names=tuple(out_names),
            lowering_input_output_aliases=(),
            sim_require_finite=True,
            sim_require_nnan=True,
            nc=nc,
        )
        return tuple(outs)

    devices = jax.devices()[:8]
    mesh = Mesh(np.asarray(devices), ("core",))
    n_outs = len(out_names)
    sharded = jax.jit(
        shard_map(
            _body,
            mesh=mesh,
            in_specs=(PartitionSpec("core"),) * (n_params + n_outs),
            out_specs=(PartitionSpec("core"),) * n_outs,
            check_rep=False,
        ),
        keep_unused=True,
    )
    runner = {
        "fn": sharded,
        "in_names": in_names,
        "out_names": out_names,
        "out_shapes": out_shapes,
    }
    _CACHE[key] = runner
    return runner


def _run_cores(in_maps, repeats=1):
    """in_maps: list of 8 dicts name->np.ndarray. Returns list of 8 output dicts."""
    r = _get_runner(repeats)
    concat_in = [
        np.concatenate([np.asarray(m[name]) for m in in_maps], axis=0)
        for name in r["in_names"]
    ]
    concat_zeros = [
        np.zeros((8 * s[0], *s[1:]), np.float32) for s in r["out_shapes"]
    ]
    out_arrs = r["fn"](*concat_in, *concat_zeros)
    outs = []
    for c in range(8):
        outs.append(
            {
                name: np.asarray(out_arrs[i]).reshape(8, *r["out_shapes"][i])[c]
                for i, name in enumerate(r["out_names"])
            }
        )
    return outs


def _make_in_maps(x, weight, bias):
    xt = np.ascontiguousarray(np.asarray(x, dtype=np.float32).T)      # [K, N_ROWS]
    wt = np.ascontiguousarray(np.asarray(weight, dtype=np.float32).T)  # [K, D_OUT]
    bias = np.asarray(bias, dtype=np.float32)
    in_maps = []
    for i in range(8):
        h, q = divmod(i, C_SHARDS)
        in_maps.append(
            {
                "xt": xt[:, h * MC : (h + 1) * MC],
                "wt": wt[:, q * NC_ : (q + 1) * NC_],
                "bias": np.broadcast_to(
                    bias[q * NC_ : (q + 1) * NC_], (P, NC_)
                ),
            }
        )
    return in_maps


def kernel(x, weight, bias):
    in_maps = _make_in_maps(x, weight, bias)
    outs = _run_cores(in_maps)
    y = np.empty((N_ROWS, D_OUT), dtype=np.float32)
    for i in range(8):
        h, q = divmod(i, C_SHARDS)
        y[h * MC : (h + 1) * MC, q * NC_ : (q + 1) * NC_] = outs[i]["y"]
    return y


# revision 4
# speedup vs baseline: 139.5396x; 1.0373x over previous
"""Block-sparse linear y = x @ W^T + b on 8 Trainium2 NeuronCores.

x: [8192, 4096] f32, W: [4096, 4096] f32 (block-sparse mask already applied),
b: [4096] f32 -> y: [8192, 4096] f32.

Sharding: 2 row-halves of x  x  4 out-feature quarters of W (8 cores).
Each core computes y_shard[4096, 1024] = x_half @ W_quarter^T + b_quarter
with fp32r matmuls (full-rate fp32 on the PE array, ~1.5e-4 rel err),
W-shard resident in SBUF, accumulating over 32 K-tiles in PSUM.
"""

import contextlib

import numpy as np
import jax
from jax.sharding import Mesh, PartitionSpec
from jax.experimental.shard_map import shard_map

import concourse.tile as tile
from concourse import bacc, mybir
from concourse.bass2jax import (
    install_neuronx_cc_hook,
    partition_id_tensor,
    _bass_exec_p,
)

P = 128
K = 4096          # contraction (in_features)
N_ROWS = 8192     # x rows
D_OUT = 4096      # out_features
R_SHARDS = 2      # row shards
C_SHARDS = 4      # out-feature shards
MC = N_ROWS // R_SHARDS    # 4096 rows per core
NC_ = D_OUT // C_SHARDS    # 1024 out features per core
KT = K // P                # 32 k-tiles
MT = MC // P               # 32 row-tiles
NT = NC_ // 512            # 2 psum-width tiles

F32 = mybir.dt.float32
F32R = mybir.dt.float32r

_CACHE = {}


def _build_nc(repeats=1):
    nc = bacc.Bacc("TRN2", target_bir_lowering=False)
    xt_d = nc.declare_dram_parameter("xt", [K, MC], F32, isOutput=False).ap()
    wt_d = nc.declare_dram_parameter("wt", [K, NC_], F32, isOutput=False).ap()
    b_d = nc.declare_dram_parameter("bias", [P, NC_], F32, isOutput=False).ap()
    y_d = nc.declare_dram_parameter("y", [MC, NC_], F32, isOutput=True).ap()

    with tile.TileContext(nc) as tc:
        with (
            tc.tile_pool(name="wpool", bufs=1) as wpool,
            tc.tile_pool(name="bpool", bufs=1) as bpool,
            tc.tile_pool(name="xpool", bufs=3) as xpool,
            tc.tile_pool(name="opool", bufs=2) as opool,
            tc.tile_pool(name="psum", bufs=4, space="PSUM") as psum,
        ):
            # resident weights [p, kt, NC_]; split the load per k-tile so
            # early matmuls can start before the whole shard arrives
            wt_sb = wpool.tile([P, KT, NC_], F32R)
            wt_src = wt_d.rearrange("(j p) n -> p j n", p=P).bitcast(F32R)
            for j in range(KT):
                nc.sync.dma_start(out=wt_sb[:, j, :], in_=wt_src[:, j, :])
            b_sb = bpool.tile([P, NC_], F32)
            nc.sync.dma_start(out=b_sb[:], in_=b_d[:])

            rep_ctx = (
                tc.For_i(0, repeats, 1, hint_engines=(mybir.EngineType.PE,))
                if repeats > 1
                else contextlib.nullcontext()
            )
            with rep_ctx:
                _emit_body(nc, tc, xpool, opool, psum, xt_d, y_d, wt_sb, b_sb)
    nc.compile()
    return nc


def _emit_body(nc, tc, xpool, opool, psum, xt_d, y_d, wt_sb, b_sb):
    for m in range(MT):
        xt_sb = xpool.tile([P, KT, P], F32R)
        nc.sync.dma_start(
            out=xt_sb[:],
            in_=xt_d[:, m * P : (m + 1) * P]
            .rearrange("(j p) f -> p j f", p=P)
            .bitcast(F32R),
        )
        o_sb = opool.tile([P, NC_], F32)
        for n in range(NT):
            ps = psum.tile([P, 512], F32)
            for j in range(KT):
                nc.tensor.matmul(
                    ps[:],
                    lhsT=xt_sb[:, j, :],
                    rhs=wt_sb[:, j, n * 512 : (n + 1) * 512],
                    start=(j == 0),
                    stop=(j == KT - 1),
                )
            nc.vector.tensor_add(
                out=o_sb[:, n * 512 : (n + 1) * 512],
                in0=ps[:],
                in1=b_sb[:, n * 512 : (n + 1) * 512],
            )
        nc.sync.dma_start(out=y_d[m * P : (m + 1) * P, :], in_=o_sb[:])


def _get_runner(repeats=1):
    """Build (once) a jitted 8-core executable: concat inputs -> concat outputs."""
    key = ("runner", repeats)
    if key in _CACHE:
        return _CACHE[key]

    install_neuronx_cc_hook()
    nc = _build_nc(repeats)

    partition_name = (
        nc.partition_id_tensor.name if nc.partition_id_tensor else None
    )
    in_names = []
    out_names = []
    out_avals = []
    out_shapes = []
    for alloc in nc.m.functions[0].allocations:
        if not isinstance(alloc, mybir.MemoryLocationSet):
            continue
        name = alloc.memorylocations[0].name
        if alloc.kind == "ExternalInput":
            if name != partition_name:
                in_names.append(name)
        elif alloc.kind == "ExternalOutput":
            shape = tuple(alloc.tensor_shape)
            out_names.append(name)
            out_shapes.append(shape)
            out_avals.append(
                jax.core.ShapedArray(shape, mybir.dt.np(alloc.dtype))
            )
    n_params = len(in_names)
    # outputs are passed as (non-donated) zero operands after the inputs
    all_names = in_names + out_names
    if partition_name is not None:
        all_names = all_names + [partition_name]

    def _body(*args):
        operands = list(args)
        if partition_name is not None:
            operands.append(partition_id_tensor())
        outs = _bass_exec_p.bind(
            *operands,
            out_avals=tuple(out_avals),
            in_names=tuple(all_names),
            out_names=tuple(out_names),
            lowering_input_output_aliases=(),
            sim_require_finite=True,
            sim_require_nnan=True,
            nc=nc,
        )
        return tuple(outs)

    devices = jax.devices()[:8]
    mesh = Mesh(np.asarray(devices), ("core",))
    n_outs = len(out_names)
    sharded = jax.jit(
        shard_map(
            _body,
            mesh=mesh,
            in_specs=(PartitionSpec("core"),) * (n_params + n_outs),
            out_specs=(PartitionSpec("core"),) * n_outs,
            check_rep=False,
        ),
        keep_unused=True,
    )
    runner = {
        "fn": sharded,
        "in_names": in_names,
        "out_names": out_names,
        "out_shapes": out_shapes,
    }
    _CACHE[key] = runner
    return runner


def _run_cores(in_maps, repeats=1):
    """in_maps: list of 8 dicts name->np.ndarray. Returns list of 8 output dicts."""
    r = _get_runner(repeats)
    concat_in = [
        np.concatenate([np.asarray(m[name]) for m in in_maps], axis=0)
        for name in r["in_names"]
    ]
    concat_zeros = [
        np.zeros((8 * s[0], *s[1:]), np.float32) for s in r["out_shapes"]
    ]
    out_arrs = r["fn"](*concat_in, *concat_zeros)
    outs = []
    for c in range(8):
        outs.append(
            {
                name: np.asarray(out_arrs[i]).reshape(8, *r["out_shapes"][i])[c]
                for i, name in enumerate(r["out_names"])
            }
        )
    return outs


def _make_in_maps(x, weight, bias):
    xt = np.ascontiguousarray(np.asarray(x, dtype=np.float32).T)      # [K, N_ROWS]
    wt = np.ascontiguousarray(np.asarray(weight, dtype=np.float32).T)  # [K, D_OUT]
    bias = np.asarray(bias, dtype=np.float32)
    in_maps = []
    for i in range(8):
        h, q = divmod(i, C_SHARDS)
        in_maps.append(
            {
                "xt": xt[:, h * MC : (h + 1) * MC],
                "wt": wt[:, q * NC_ : (q + 1) * NC_],
                "bias": np.broadcast_to(
                    bias[q * NC_ : (q + 1) * NC_], (P, NC_)
                ),
            }
        )
    return in_maps


def kernel(x, weight, bias):
    in_maps = _make_in_maps(x, weight, bias)
    outs = _run_cores(in_maps)
    y = np.empty((N_ROWS, D_OUT), dtype=np.float32)
    for i in range(8):
        h, q = divmod(i, C_SHARDS)
        y[h * MC : (h + 1) * MC, q * NC_ : (q + 1) * NC_] = outs[i]["y"]
    return y
